# revision 5
# baseline (speedup 1.0000x reference)
"""Compact bilinear pooling kernel for 8 Trainium2 NeuronCores (v2).

Algorithm (host folds count-sketch + FFT into dense matmul weights):
  out[b,:,n] = circconv_1024(S1 @ x1[b,:,n], S2 @ x2[b,:,n])
via x^1024-1 = (x^512-1)(x^512+1): cyclic-512 (level-2 folded rFFT) +
negacyclic-512 branches fused into W_j [512c -> 1024 freq rows] bf16
matmuls; complex multiply on DVE via 4x-mode TensorScalarPtr ops; inverse
is block-diagonal bf16 matmuls (IE/IF/ID) with unfolds on DVE.

v2 layout: everything bf16 end-to-end (host casts), positions flattened
to [C, 3136] per core, PT=448 (7 tiles), one DMA per (input,tile) via
[128, 4, 3136] DRAM layout, one store DMA per tile via [128, 8, 448]
staging, loads on Pool queue / stores+weights on SP queue.

Sharding: batch 32 -> 4 per core (data parallel), weights replicated.
"""
import sys

sys.path.insert(0, "/opt/trn_rl_repo")

import numpy as np
import ml_dtypes
import concourse.bass as bass
import concourse.mybir as mybir
from concourse import bacc
from concourse.tile import TileContext
from concourse.bass_utils import run_bass_kernel_spmd

B, C, HW, O = 32, 512, 784, 1024
NCORES = 8
BPC = B // NCORES  # 4 batches per core
N = BPC * HW  # 3136 positions per core
PT = 448
NT = N // PT  # 7
H = O // 2  # 512
F32, BF16 = mybir.dt.float32, mybir.dt.bfloat16
MULT = mybir.AluOpType.mult
ADD = mybir.AluOpType.add
SUB = mybir.AluOpType.subtract
BF = ml_dtypes.bfloat16


def _build_host_matrices(sketch1, sketch2):
    """Fused fwd [512 c, 1024 freq-rows]; inverse IE/IF [256,256], ID [512,512].

    Level-2 folded row layout: e=rfft256, f=oddDFT256, d=oddDFT512; inverse
    weights carry the unfold 1/2 factors.
    """

    def build_fwd(sketch):
        sk = np.asarray(sketch, dtype=np.float64)
        Sp = sk[:H] + sk[H:]
        Sm = sk[:H] - sk[H:]
        Spp = Sp[:256] + Sp[256:]
        Spm = Sp[:256] - Sp[256:]
        n2 = np.arange(256)[None, :]
        k2 = np.arange(129)[:, None]
        Mc2 = np.exp(-2j * np.pi * k2 * n2 / 256) @ Spp
        k2f = np.arange(128)[:, None]
        Mo2 = np.exp(-2j * np.pi * n2 * (2 * k2f + 1) / 512) @ Spm
        n = np.arange(H)[None, :]
        ko = np.arange(256)[:, None]
        Mo = np.exp(-2j * np.pi * n * (2 * ko + 1) / O) @ Sm
        W = np.zeros((O, C))
        W[0:128] = Mc2[0:128].real
        W[128] = Mc2[128].real
        W[129:256] = Mc2[1:128].imag
        W[256:384] = Mo2.real
        W[384:512] = Mo2.imag
        W[512:768] = Mo.real
        W[768:1024] = Mo.imag
        return np.ascontiguousarray(W.T).astype(np.float32)  # [C, O]

    j2 = np.arange(256)[None, :]
    k = np.arange(128)[:, None]
    IE = np.zeros((256, 256))
    IE[0:128] = 2 * np.cos(2 * np.pi * k * j2 / 256) / 256
    IE[0] = 1.0 / 256
    IE[128] = np.cos(np.pi * j2) / 256
    ki = np.arange(1, 128)[:, None]
    IE[129:256] = -2 * np.sin(2 * np.pi * ki * j2 / 256) / 256
    IF = np.zeros((256, 256))
    IF[0:128] = 2 * np.cos(2 * np.pi * (2 * k + 1) * j2 / 512) / 256
    IF[128:256] = -2 * np.sin(2 * np.pi * (2 * k + 1) * j2 / 512) / 256
    j = np.arange(H)[None, :]
    ko = np.arange(256)[:, None]
    ID = np.zeros((H, H))
    ID[0:256] = 2 * np.cos(2 * np.pi * (2 * ko + 1) * j / O) / H
    ID[256:512] = -2 * np.sin(2 * np.pi * (2 * ko + 1) * j / O) / H
    return (
        build_fwd(sketch1),
        build_fwd(sketch2),
        (IE / 4).astype(np.float32),
        (IF / 4).astype(np.float32),
        (ID / 2).astype(np.float32),
    )


def _chunked(mat, nch, width):
    """[nch*128, width] -> [128, nch, width] bf16 (channel chunk on free dim)."""
    return np.ascontiguousarray(
        mat.reshape(nch, 128, width).transpose(1, 0, 2)
    ).astype(BF)


def _build_program(cfg=None):
    cfg = cfg or {}
    psf_bufs = cfg.get("psf_bufs", 3)
    pse_bufs = cfg.get("pse_bufs", 1)
    psq_bufs = cfg.get("psq_bufs", 2)
    psd_bufs = cfg.get("psd_bufs", 2)
    xbufs = cfg.get("xbufs", 3)
    fbufs = cfg.get("fbufs", 2)
    obufs = cfg.get("obufs", 2)
    sbufs = cfg.get("sbufs", 2)
    xqueue = cfg.get("xqueue", "gpsimd")  # gpsimd | sync | scalar

    nc = bacc.Bacc(None)
    x1e = nc.declare_dram_parameter("x1", [128, 4, N], BF16, isOutput=False)
    x2e = nc.declare_dram_parameter("x2", [128, 4, N], BF16, isOutput=False)
    w1e = nc.declare_dram_parameter("w1", [128, 4, O], BF16, isOutput=False)
    w2e = nc.declare_dram_parameter("w2", [128, 4, O], BF16, isOutput=False)
    iee = nc.declare_dram_parameter("ie", [128, 2, 256], BF16, isOutput=False)
    ife = nc.declare_dram_parameter("if", [128, 2, 256], BF16, isOutput=False)
    ide = nc.declare_dram_parameter("id", [128, 4, H], BF16, isOutput=False)
    oute = nc.declare_dram_parameter("out", [128, 8, N], BF16, isOutput=True)

    with TileContext(nc) as tc:
        with (
            tc.tile_pool(name="wpool", bufs=1) as wpool,
            tc.tile_pool(name="xpool", bufs=xbufs) as xpool,
            tc.tile_pool(name="fpool", bufs=fbufs) as fpool,
            tc.tile_pool(name="opool", bufs=obufs) as opool,
            tc.tile_pool(name="spool", bufs=sbufs) as spool,
            tc.tile_pool(name="psf", bufs=psf_bufs, space="PSUM") as psf,
            tc.tile_pool(name="pse", bufs=pse_bufs, space="PSUM") as pse,
            tc.tile_pool(name="psq", bufs=psq_bufs, space="PSUM") as psq,
            tc.tile_pool(name="psd", bufs=psd_bufs, space="PSUM") as psd,
        ):
            xeng = {"gpsimd": nc.gpsimd, "sync": nc.sync, "scalar": nc.scalar}[
                xqueue
            ]
            cs_eng = {"vector": nc.vector, "gpsimd": nc.gpsimd}[
                cfg.get("cs_eng", "vector")
            ]
            # per-oc engine for the final unfold pair (lo, hi)
            unf_engs = [
                {"vector": nc.vector, "gpsimd": nc.gpsimd}[e]
                for e in cfg.get("unf_engs", ["gpsimd", "gpsimd", "gpsimd", "gpsimd"])
            ]

            def load_x(t, nsl):
                xts = []
                for j, xe in ((1, x1e), (2, x2e)):
                    xt = xpool.tile([128, 4, PT], BF16, tag=f"x{j}", name=f"x{j}_{t}")
                    xeng.dma_start(out=xt[:], in_=xe[:, :, nsl])
                    xts.append(xt)
                return xts

            # ---- startup: spread issue across queues; w1 split per chunk so
            # the first j=1 matmul group starts as soon as (x1 t0, w1 cc0) land
            x1t0 = xpool.tile([128, 4, PT], BF16, tag="x1", name="x1_0")
            w1t = wpool.tile([128, 4, O], BF16, name="w1t")
            w2t = wpool.tile([128, 4, O], BF16, name="w2t")
            x2t0 = xpool.tile([128, 4, PT], BF16, tag="x2", name="x2_0")
            nc.gpsimd.dma_start(out=x1t0[:, 0, :], in_=x1e[:, 0, 0:PT])
            nc.sync.dma_start(out=w1t[:, 0, :], in_=w1e[:, 0, :])
            nc.gpsimd.dma_start(out=x1t0[:, 1:4, :], in_=x1e[:, 1:4, 0:PT])
            nc.sync.dma_start(out=w1t[:, 1:4, :], in_=w1e[:, 1:4, :])
            x2t0_eng = {"gpsimd": nc.gpsimd, "scalar": nc.scalar}[
                cfg.get("x2t0_q", "scalar")
            ]
            x2t0_eng.dma_start(out=x2t0[:], in_=x2e[:, :, 0:PT])
            nc.sync.dma_start(out=w2t[:], in_=w2e[:])
            x_pre = {0: (x1t0, x2t0)}
            iet = wpool.tile([128, 2, 256], BF16, name="iet")
            nc.scalar.dma_start(out=iet[:], in_=iee[:])
            ift = wpool.tile([128, 2, 256], BF16, name="ift")
            nc.scalar.dma_start(out=ift[:], in_=ife[:])
            idt = wpool.tile([128, 4, H], BF16, name="idt")
            nc.scalar.dma_start(out=idt[:], in_=ide[:])

            stt = nc.vector.scalar_tensor_tensor

            # ---- PE warmup: matmuls on a zeroed tile while the first real
            # DMAs land; also brings the PE out of low p-state before real work
            n_warm = cfg.get("n_warm", 20)
            warm_w = cfg.get("warm_w", 128)
            if n_warm:
                zt = wpool.tile([128, warm_w], BF16, name="zwarm")
                nc.vector.memset(zt[:], 0.0)
                for wi in range(n_warm):
                    pz = psf.tile([128, warm_w], F32, tag="psf", name=f"warm{wi}")
                    nc.tensor.matmul(
                        pz[:], zt[:, 0:128], zt[:], start=True, stop=True
                    )

            fc_order = cfg.get("fc_order", [4, 5, 6, 7, 0, 1, 2, 3])
            pair_order = cfg.get("pair_order", [(4, 6), (5, 7), (0, 1), (2, 3)])
            groups_inter = [(j, fc) for fc in fc_order for j in (1, 2)]
            # tile 0: all x1 groups first (x2/w2 DMAs still in flight)
            groups_jmaj = [(1, fc) for fc in fc_order] + [(2, fc) for fc in fc_order]

            # jobs: (tag, x-load tile idx or None, n0, pw, groups)
            jobs = [(t, t * PT, PT, groups_jmaj if t == 0 else groups_inter)
                    for t in range(NT)]
            tail_mode = cfg.get("tail_mode", "hhq")
            if tail_mode == "hh":
                t, n0, pw, go = jobs.pop()
                jobs.append((t, n0, PT // 2, go))
                jobs.append((t + 1, n0 + PT // 2, PT // 2, go))
            elif tail_mode == "hhq":
                t, n0, pw, go = jobs.pop()
                jobs.append((t, n0, PT // 2, go))
                jobs.append((t + 1, n0 + PT // 2, PT // 4, go))
                jobs.append((t + 2, n0 + 3 * PT // 4, PT // 4, go))

            loaded = {}
            for t, n0, pw, groups in jobs:
                last = t >= NT - 1
                nsl = slice(n0, n0 + pw)
                if t == 0:
                    x1t, x2t = x_pre.pop(0)
                elif t > NT - 1 and NT - 1 in loaded:
                    x1t, x2t = loaded[NT - 1]  # split tail shares the load
                else:
                    x1t, x2t = load_x(t, slice(n0, n0 + PT))
                    loaded[t] = (x1t, x2t)

                # forward: fft[(j, fc)] [128, PT] bf16; d-chunks (4-7) first,
                # x1/x2 interleaved so cmul pairs complete early
                fft = {}
                xof = n0 - (NT - 1) * PT if t > NT - 1 else 0  # tail offsets
                for j, fc in groups:
                    wt = w1t if j == 1 else w2t
                    xt = x1t if j == 1 else x2t
                    ps = psf.tile([128, PT], F32, tag="psf", name=f"psf{j}_{fc}_{t}")
                    osl = slice(fc * 128, (fc + 1) * 128)
                    for cc in range(4):
                        nc.tensor.matmul(
                            ps[:, :pw],
                            wt[:, cc, osl],
                            xt[:, cc, xof : xof + pw],
                            start=(cc == 0),
                            stop=(cc == 3),
                        )
                    ft = fpool.tile(
                        [128, PT], BF16, tag=f"fft{j}_{fc}", name=f"fft{j}_{fc}_{t}"
                    )
                    nc.scalar.copy(out=ft[:, :pw], in_=ps[:, :pw])
                    fft[(j, fc)] = ft

                # complex multiply on DVE (bf16 tensor_tensor, 2x mode)
                W_ = slice(0, pw)
                prod = {}
                for re_c, im_c in pair_order:
                    a1, b1 = fft[(1, re_c)], fft[(1, im_c)]
                    a2, b2 = fft[(2, re_c)], fft[(2, im_c)]
                    m1 = fpool.tile([128, PT], BF16, tag="m1", name=f"m1_{re_c}_{t}")
                    m2 = fpool.tile([128, PT], BF16, tag="m2", name=f"m2_{re_c}_{t}")
                    pr = fpool.tile([128, PT], BF16, tag=f"pr{re_c}", name=f"pr{re_c}_{t}")
                    pi = fpool.tile([128, PT], BF16, tag=f"pi{im_c}", name=f"pi{im_c}_{t}")
                    eng = nc.vector
                    eng.tensor_mul(m1[:, W_], a1[:, W_], a2[:, W_])
                    eng.tensor_mul(m2[:, W_], b1[:, W_], b2[:, W_])
                    eng.tensor_sub(pr[:, W_], m1[:, W_], m2[:, W_])
                    eng.tensor_mul(m1[:, W_], a1[:, W_], b2[:, W_])
                    eng.tensor_mul(m2[:, W_], b1[:, W_], a2[:, W_])
                    eng.tensor_add(pi[:, W_], m1[:, W_], m2[:, W_])
                    if re_c == 0:
                        # row 0 of (0,1): DC_e (re) and Nyquist-256 (im slot)
                        # are real-only products
                        eng.tensor_mul(pr[0:1, W_], a1[0:1, W_], a2[0:1, W_])
                        eng.tensor_mul(pi[0:1, W_], b1[0:1, W_], b2[0:1, W_])
                    prod[re_c] = pr
                    prod[im_c] = pi

                # inverse d first (products ready earliest); drain each psd
                # bank to SBUF through Act so banks recycle fast
                ds = []
                for oc in range(4):
                    osl = slice(oc * 128, (oc + 1) * 128)
                    pd = psd.tile([128, PT], F32, tag="psd", name=f"psd{oc}_{t}")
                    for rc in range(4):
                        nc.tensor.matmul(
                            pd[:, :pw], idt[:, rc, osl], prod[4 + rc][:, :pw],
                            start=(rc == 0), stop=(rc == 3),
                        )
                    dt_ = opool.tile([128, PT], F32, tag=f"ds{oc}", name=f"ds{oc}_{t}")
                    nc.scalar.copy(out=dt_[:, :pw], in_=pd[:, :pw])
                    ds.append(dt_)

                # inverse level2: e,f [256] then c = unfold2(e,f)
                cch = []
                for oc2 in range(2):
                    osl2 = slice(oc2 * 128, (oc2 + 1) * 128)
                    pe_ = pse.tile([128, PT], F32, tag="pse", name=f"pse{oc2}_{t}")
                    pf_ = psq.tile([128, PT], F32, tag="psq", name=f"psq{oc2}_{t}")
                    for rc in range(2):
                        nc.tensor.matmul(
                            pe_[:, :pw], iet[:, rc, osl2], prod[rc][:, :pw],
                            start=(rc == 0), stop=(rc == 1),
                        )
                    for rc in range(2):
                        nc.tensor.matmul(
                            pf_[:, :pw], ift[:, rc, osl2], prod[2 + rc][:, :pw],
                            start=(rc == 0), stop=(rc == 1),
                        )
                    es = opool.tile([128, PT], F32, tag=f"es{oc2}", name=f"es{oc2}_{t}")
                    nc.scalar.copy(out=es[:, :pw], in_=pe_[:, :pw])
                    cch.append((es, pf_))
                cs = []
                for oc in range(4):
                    es, pf_ = cch[oc % 2]
                    ct = opool.tile([128, PT], F32, tag=f"c{oc}", name=f"c{oc}_{t}")
                    ce = cs_eng
                    if last and cfg.get("ltail") == "alt" and oc % 2 == 1:
                        ce = nc.gpsimd
                    if oc < 2:
                        ce.tensor_add(ct[:, :pw], es[:, :pw], pf_[:, :pw])
                    else:
                        ce.tensor_sub(ct[:, :pw], es[:, :pw], pf_[:, :pw])
                    cs.append(ct)

                # final unfold into bf16 staging
                stage = spool.tile([128, 8, PT], BF16, tag="stage", name=f"stage_{t}")
                if last:
                    # tail: unfold slot pairs and store each pair as soon as
                    # it is ready — shortens the final drain chain
                    vlast = t == jobs[-1][0]
                    tmode = cfg.get("ltail", "dve")
                    tails = (
                        [nc.vector] * 4
                        if (vlast and tmode == "dve")
                        else [nc.vector, nc.gpsimd, nc.vector, nc.gpsimd]
                    )
                    for oc in range(4):
                        tails[oc].tensor_add(
                            stage[:, oc, :pw], cs[oc][:, :pw], ds[oc][:, :pw]
                        )
                        if oc % 2 == 1:
                            nc.sync.dma_start(
                                out=oute[:, oc - 1 : oc + 1, nsl],
                                in_=stage[:, oc - 1 : oc + 1, :pw],
                            )
                    for oc in range(4):
                        tails[oc].tensor_sub(
                            stage[:, 4 + oc, :pw], cs[oc][:, :pw], ds[oc][:, :pw]
                        )
                        if oc % 2 == 1:
                            nc.sync.dma_start(
                                out=oute[:, 3 + oc : 5 + oc, nsl],
                                in_=stage[:, 3 + oc : 5 + oc, :pw],
                            )
                else:
                    for oc in range(4):
                        ue = unf_engs[oc]
                        ue.tensor_add(stage[:, oc, :pw], cs[oc][:, :pw], ds[oc][:, :pw])
                        ue.tensor_sub(
                            stage[:, 4 + oc, :pw], cs[oc][:, :pw], ds[oc][:, :pw]
                        )
                    nc.sync.dma_start(out=oute[:, :, nsl], in_=stage[:, :, :pw])

    nc.finalize()
    return nc


_NC_CACHE = None
KCFG = {"tail_mode": "hh", "n_warm": 16, "fbufs": 3, "x2t0_q": "gpsimd", "ltail": "dve"}


def _make_in_maps(x1, x2, sketch1, sketch2):
    w1, w2, ie, if_, idm = _build_host_matrices(sketch1, sketch2)
    w1c = _chunked(w1, 4, O)
    w2c = _chunked(w2, 4, O)
    iec = _chunked(ie, 2, 256)
    ifc = _chunked(if_, 2, 256)
    idc = _chunked(idm, 4, H)
    # [B, C, H, W] -> per core [C, BPC, HW] -> [128, 4cc, N] bf16
    x1f = np.asarray(x1, dtype=np.float32).reshape(B, C, HW)
    x2f = np.asarray(x2, dtype=np.float32).reshape(B, C, HW)
    in_maps = []
    for i in range(NCORES):
        bs = slice(i * BPC, (i + 1) * BPC)
        maps = {"w1": w1c, "w2": w2c, "ie": iec, "if": ifc, "id": idc}
        for name, xf in (("x1", x1f), ("x2", x2f)):
            xc = xf[bs].transpose(1, 0, 2).reshape(C, N)  # [C, N]
            maps[name] = np.ascontiguousarray(
                xc.reshape(4, 128, N).transpose(1, 0, 2)
            ).astype(BF)
        in_maps.append(maps)
    return in_maps


def _unshard_out(results):
    outs = []
    for i in range(NCORES):
        oc = np.asarray(results[i]["out"]).astype(np.float32)  # [128, 8, N]
        oc = oc.transpose(1, 0, 2).reshape(O, BPC, HW).transpose(1, 0, 2)
        outs.append(oc)
    return np.concatenate(outs, axis=0).reshape(B, O, 28, 28)


def kernel(x1, x2, sketch1, sketch2):
    global _NC_CACHE
    if _NC_CACHE is None:
        _NC_CACHE = _build_program(KCFG)
    in_maps = _make_in_maps(x1, x2, sketch1, sketch2)
    res = run_bass_kernel_spmd(_NC_CACHE, in_maps, list(range(NCORES)))
    return _unshard_out(res.results)


if __name__ == "__main__":
    rng = np.random.default_rng(0)
    x1 = rng.standard_normal((B, C, 28, 28)).astype(np.float32)
    x2 = rng.standard_normal((B, C, 28, 28)).astype(np.float32)
    h1 = rng.integers(0, O, C)
    s1 = rng.integers(0, 2, C) * 2.0 - 1.0
    h2 = rng.integers(0, O, C)
    s2 = rng.integers(0, 2, C) * 2.0 - 1.0
    sk1 = np.zeros((O, C), np.float32)
    sk1[h1, np.arange(C)] = s1
    sk2 = np.zeros((O, C), np.float32)
    sk2[h2, np.arange(C)] = s2
    got = kernel(x1, x2, sk1, sk2)
    p1 = np.einsum("bchw,oc->bohw", x1, sk1).reshape(B, O, HW)
    p2 = np.einsum("bchw,oc->bohw", x2, sk2).reshape(B, O, HW)
    ref = np.fft.ifft(np.fft.fft(p1, axis=1) * np.fft.fft(p2, axis=1), axis=1).real
    err = np.abs(got.reshape(B, O, HW) - ref).max() / np.abs(ref).max()
    print("self-test max rel err:", err)
